# revision 28
# baseline (speedup 1.0000x reference)
"""C2Q attention kernel for 8 TRN2 NeuronCores, ragged-aware.

Math (per batch, bias dropped — a per-i constant shift cancels in the
j-softmax; masked entries are the same constant NEG in reference either way):
    score[i,j] = (oc @ W @ oq.T)[i,j] / sqrt(H)
    e[j,i]     = exp(score/32 + qb[j]),  qb in {0, -60000} masks j >= q_len
    out[i,:]   = (e.T @ oq[:qs*128]) / sum_j e[j,i],  rows i >= c_len dropped

Ragged slotting: each batch needs only qt = ceil(q_len/128) j-tiles and
ct = ceil(c_len/128) i-tiles of compute.  kernel() quantizes the actual
lengths, picks 4 slot structures (qs, cs) that cover the 32-batch multiset
(one batch per core per slot, min total matmul rows, small max-flow), and
compiles a program specialized to those structures.  Each slot also picks
the cheaper association order for the H x H Linear:
    q-side: u = W @ oq.T   (cost ~ qs), score = u.T @ ocT
    c-side: v = W.T @ oc.T (cost ~ cs), score = oqT.T @ v
Everything lands K-on-partitions with zero on-chip transposes; the
denominator is a ones-row matmul partition-reduce shipped to the host,
which does the 1/d normalization (frees the PE/DVE transpose chain and the
tail dependency).  Output ships fp16 unnormalized (harness tolerance 2e-2;
fp16 adds ~1e-4; |ctx_raw| and d both well inside fp16/f32 range).
"""

import os
import sys

import numpy as np

if "/opt/trn_rl_repo" not in sys.path:
    sys.path.insert(0, "/opt/trn_rl_repo")

B, Tc, Tq, H = 32, 512, 512, 1024
N_CORES = 8
B_LOCAL = B // N_CORES  # 4 slots, one batch per core per slot
KT = H // 128  # contraction tiles over h (8)
SCALE = 1.0 / 32.0  # 1/sqrt(H)
NEG16 = -60000.0  # exp(x - 60000) == 0 exactly in fp32


def _unit_cost(q, c):
    # matmul rows in units of 128: Linear 64*min + score 8qc + dmm qc + ctx 8qc
    return 64 * min(q, c) + 17 * q * c


def _plan_slots(qt, ct):
    """Pick 4 slot structures covering {(qt_i, ct_i)} with 8 batches each,
    minimizing total matmul rows.  Returns (slots_in_exec_order,
    assignment) where assignment[b] = (core, slot)."""
    from itertools import combinations_with_replacement
    import collections

    qmax, cmax = int(max(qt)), int(max(ct))
    types = collections.Counter(zip(map(int, qt), map(int, ct)))
    tlist = list(types.items())  # [((q,c), count)]

    def feasible(combo):
        # max-flow on aggregated graph: type -> slot (cap 8 per slot)
        cap = {}  # (u,v) residual
        SRC, SNK = "S", "T"
        for i, ((q, c), n) in enumerate(tlist):
            cap[(SRC, i)] = n
        for s, (qs, cs) in enumerate(combo):
            cap[(("s", s), SNK)] = N_CORES
            for i, ((q, c), n) in enumerate(tlist):
                if q <= qs and c <= cs:
                    cap[(i, ("s", s))] = n
        flow = 0
        while True:
            # BFS augment
            prev = {SRC: None}
            queue = [SRC]
            while queue and SNK not in prev:
                u = queue.pop(0)
                for (a, b), r in cap.items():
                    if a == u and r > 0 and b not in prev:
                        prev[b] = u
                        queue.append(b)
            if SNK not in prev:
                break
            # min residual along path
            path = []
            v = SNK
            while prev[v] is not None:
                path.append((prev[v], v))
                v = prev[v]
            aug = min(cap[e] for e in path)
            for e in path:
                cap[e] -= aug
                cap[(e[1], e[0])] = cap.get((e[1], e[0]), 0) + aug
            flow += aug
        if flow < B:
            return None
        # extract assignment counts: type i -> slot s
        out = {}
        for (a, b), r in cap.items():
            if isinstance(b, int) and isinstance(a, tuple) and a[0] == "s":
                if r > 0:
                    out[(b, a[1])] = r  # reverse edge = flow
        return out

    structs = [(q, c) for q in range(1, qmax + 1) for c in range(1, cmax + 1)]
    best = None
    for combo in combinations_with_replacement(structs, B_LOCAL):
        cost = sum(_unit_cost(q, c) for q, c in combo)
        if best is not None and cost >= best[0]:
            continue
        asn = feasible(combo)
        if asn is not None:
            best = (cost, combo, asn)
    cost, combo, asn = best
    # exec order: biggest first; last slot prefers small cs (out-DMA tail)
    order = sorted(range(B_LOCAL), key=lambda s: (-_unit_cost(*combo[s]), -combo[s][1]))
    slots = [combo[s] for s in order]
    # distribute batches: slot (exec index) x 8 cores
    per_slot = {s: [] for s in range(B_LOCAL)}
    bidx_by_type = {}
    for b in range(B):
        bidx_by_type.setdefault((int(qt[b]), int(ct[b])), []).append(b)
    for (i, s_orig), n in asn.items():
        s = order.index(s_orig)
        for _ in range(n):
            per_slot[s].append(bidx_by_type[tlist[i][0]].pop())
    assignment = {}
    for s in range(B_LOCAL):
        assert len(per_slot[s]) == N_CORES
        for c, b in enumerate(per_slot[s]):
            assignment[b] = (c, s)
    return slots, assignment


def _build_program(slots):
    import concourse.bacc as bacc
    import concourse.mybir as mybir
    import concourse.tile as tile

    f32 = mybir.dt.float32
    f16 = mybir.dt.float16
    Copy = mybir.ActivationFunctionType.Copy
    Exp = mybir.ActivationFunctionType.Exp

    sides = ["q" if qs <= cs else "c" for qs, cs in slots]
    use_q = "q" in sides
    use_c = "c" in sides

    nc = bacc.Bacc("TRN2", debug=False)

    # DRAM params.  W slabs are [128, 8*1024] with per-outtile column groups:
    #   wtQ[p, o*1024 + k*128 + c] = W[o*128+c, k*128+p]   (lhsT for q-side)
    #   wtC[p, g*1024 + o*128 + c] = W[o*128+p, g*128+c]   (lhsT for c-side)
    wtQ_d = nc.declare_dram_parameter("wtQ", [128, KT * H], f16, isOutput=False) if use_q else None
    wtC_d = nc.declare_dram_parameter("wtC", [128, KT * H], f16, isOutput=False) if use_c else None
    oqT_d, ocT_d, oqN_d, out_d, den_d = [], [], [], [], []
    for s, (qs, cs) in enumerate(slots):
        KW, CW = qs * 128 + 8, cs * 128 + 8
        oqT_d.append(nc.declare_dram_parameter(f"oqT{s}", [128, KT * KW], f16, isOutput=False))
        ocT_d.append(nc.declare_dram_parameter(f"ocT{s}", [128, KT * CW], f16, isOutput=False))
        oqN_d.append(nc.declare_dram_parameter(f"oqN{s}", [128, qs, H], f16, isOutput=False))
        out_d.append(nc.declare_dram_parameter(f"out{s}", [cs * 128, H], f16, isOutput=True))
        den_d.append(nc.declare_dram_parameter(f"den{s}", [1, cs * 128], f32, isOutput=True))

    with tile.TileContext(nc) as tc:
        with (
            tc.tile_pool(name="const", bufs=1) as cpool,
            tc.tile_pool(name="inp", bufs=1) as ipool,
            tc.tile_pool(name="work", bufs=1) as wpool,
            tc.tile_pool(name="outp", bufs=4) as opool,
            tc.tile_pool(name="ps_u", bufs=3, space="PSUM") as ps_u,
            tc.tile_pool(name="ps_s", bufs=2, space="PSUM") as ps_s,
            tc.tile_pool(name="ps_c", bufs=3, space="PSUM") as ps_c,
        ):
            wtQ = cpool.tile([128, KT * H], f16, tag="wtQ", name="wtQ") if use_q else None
            wtC = cpool.tile([128, KT * H], f16, tag="wtC", name="wtC") if use_c else None

            oqT_t, ocT_t, oqN_t = [], [], []
            for s, (qs, cs) in enumerate(slots):
                KW, CW = qs * 128 + 8, cs * 128 + 8
                oqT_t.append(ipool.tile([128, KT * KW], f16, tag=f"oqT{s}", name=f"oqT{s}"))
                ocT_t.append(ipool.tile([128, KT * CW], f16, tag=f"ocT{s}", name=f"ocT{s}"))
                oqN_t.append(ipool.tile([128, qs, H], f16, tag=f"oqN{s}", name=f"oqN{s}"))

            # ---- DMA staging.  Rings execute descriptors in emission
            # order and share ~310 GB/s of HBM read bandwidth, so slot-0's
            # pieces are emitted in compute-need order, alternating between
            # the two HWDGE rings (sync/scalar) to approximate one
            # need-ordered stream at full rate.  Slot s >= 1 inputs are
            # emitted from inside slot s-1's compute (prefetch depth 1):
            # oqT on sync, ocT/oqN on scalar. ----
            lin_is_q = sides[0] == "q"
            wt0 = wtQ if lin_is_q else wtC
            wt0_d = wtQ_d if lin_is_q else wtC_d
            wt1 = wtC if lin_is_q else wtQ
            wt1_d = wtC_d if lin_is_q else wtQ_d
            lin_t = oqT_t[0] if lin_is_q else ocT_t[0]
            lin_d = oqT_d[0] if lin_is_q else ocT_d[0]
            sc_t = ocT_t[0] if lin_is_q else oqT_t[0]
            sc_d = ocT_d[0] if lin_is_q else oqT_d[0]
            LW, SW = lin_t.shape[1], sc_t.shape[1]
            LQ = LW // 4
            qn0 = max(1, slots[0][0] // 2)
            # the PE clock ramps over ~3us of continuous execution; burn
            # dummy matmuls during the 6-10us DMA wait so the first real
            # matmuls start at full clock
            warm_sb = cpool.tile([128, 512], f16, tag="warmsb", name="warmsb")
            nc.vector.memset(warm_sb, 0.125)
            warm_ps = ps_c.tile([128, 512], f32, tag="cps", name="warm_ps")
            for _ in range(14):
                nc.tensor.matmul(
                    warm_ps, warm_sb[:, :128], warm_sb[:, :],
                    start=True, stop=True, skip_group_check=True,
                )

            pieces = []
            pieces.append((wt0[:, : H // 2], wt0_d[:, : H // 2]))
            pieces.append((wt0[:, H // 2 : H], wt0_d[:, H // 2 : H]))
            pieces.append((wt0[:, H : 2 * H], wt0_d[:, H : 2 * H]))
            for i in range(4):
                pieces.append((lin_t[:, i * LQ : (i + 1) * LQ], lin_d[:, i * LQ : (i + 1) * LQ]))
            for o in range(2, KT):
                pieces.append((wt0[:, o * H : (o + 1) * H], wt0_d[:, o * H : (o + 1) * H]))
            pieces.append((sc_t[:, : SW // 2], sc_d[:, : SW // 2]))
            pieces.append((sc_t[:, SW // 2 :], sc_d[:, SW // 2 :]))
            pieces.append((oqN_t[0][:, :qn0, :], oqN_d[0][:, :qn0, :]))
            pieces.append((oqN_t[0][:, qn0:, :], oqN_d[0][:, qn0:, :]))
            for idx, (dst, srcap) in enumerate(pieces):
                eng = nc.sync if idx % 2 == 0 else nc.scalar
                eng.dma_start(out=dst, in_=srcap)

            # Queues race ahead of compute, so an input DMA only waits if it
            # sits behind an instruction with a real dependency on its queue
            # (an out-DMA gated on an evict, or the den row gated on its
            # copy).  s >= 2 inputs sit behind slot s-2's outs on both
            # queues; slot 1's ocT/oqN are deferred to slot-0 ctx time so
            # only oqT_s1 shares the ramp window with slot-0's stream.
            def emit_inputs(s):
                if s == 2 and wt1 is not None and wt1 is not wt0:
                    nc.sync.dma_start(out=wt1, in_=wt1_d[:, :])
                nc.sync.dma_start(out=oqT_t[s], in_=oqT_d[s][:, :])
                if s >= 2:
                    nc.sync.dma_start(out=ocT_t[s], in_=ocT_d[s][:, :])
                    nc.sync.dma_start(out=oqN_t[s], in_=oqN_d[s][:, :, :])

            def emit_late_inputs(s):
                nc.scalar.dma_start(out=ocT_t[s], in_=ocT_d[s][:, :])
                nc.scalar.dma_start(out=oqN_t[s], in_=oqN_d[s][:, :, :])

            # ---- compute, slot by slot ----
            for s, (qs, cs) in enumerate(slots):
                KW, CW = qs * 128 + 8, cs * 128 + 8
                side = sides[s]
                m = qs if side == "q" else cs
                oqT, ocT, oqN = oqT_t[s], ocT_t[s], oqN_t[s]
                wt = wtQ if side == "q" else wtC

                # Linear: u[o, j] (q-side) or v[g, i] (c-side), fp16 in SBUF
                u = wpool.tile([128, KT, m * 128], f16, tag=f"u{s}", name=f"u{s}")
                rhs_full = (
                    lambda k: oqT[:, k * KW : k * KW + qs * 128]
                ) if side == "q" else (
                    lambda k: ocT[:, k * CW : k * CW + cs * 128]
                )
                def lin_mm(ups, o, k):
                    nc.tensor.matmul(
                        ups,
                        wt[:, o * H + k * 128 : o * H + (k + 1) * 128],
                        rhs_full(k),
                        start=(k == 0),
                        stop=(k == KT - 1),
                    )

                ups_t = {}
                for o in range(KT):
                    if s == 0 and o == 0:
                        # ramp: run o0-o3 as a quad of paired k-pairs
                        # (borrowing the idle score-psum pair) so each
                        # arriving Linear-rhs quarter unlocks 4x the work
                        for oo in range(4):
                            pool = ps_u if oo < 2 else ps_s
                            ups_t[oo] = pool.tile(
                                [128, m * 128], f32, tag="ups" if oo < 2 else "sps",
                                name=f"ups{s}_{oo}",
                            )
                        for kq in range(4):
                            for oo in range(4):
                                for k in range(2 * kq, 2 * kq + 2):
                                    lin_mm(ups_t[oo], oo, k)
                        for oo in range(4):
                            nc.scalar.activation(out=u[:, oo, :], in_=ups_t[oo], func=Copy)
                        continue
                    if s == 0 and o < 4:
                        continue
                    ups_t[o] = ps_u.tile(
                        [128, m * 128], f32, tag="ups", name=f"ups{s}_{o}"
                    )
                    for k in range(KT):
                        lin_mm(ups_t[o], o, k)
                    nc.scalar.activation(out=u[:, o, :], in_=ups_t[o], func=Copy)

                # score + exp: e[j, i] tiles; denominator accumulates one
                # tile behind so its chain latency hides
                ones = ocT[:, cs * 128 : cs * 128 + 1]
                # dps borrows the ctx pool, which is idle through the score
                # phase; its consumer (the dsb copy) runs before the second
                # ctx rotation needs the slot back
                dps = ps_c.tile([1, cs * 128], f32, tag="cps", name=f"dps_{s}")
                e_tiles = []
                for jt in range(qs):
                    sps = ps_s.tile([128, cs * 128], f32, tag="sps", name=f"sps{s}_{jt}")
                    for z in range(KT):
                        if side == "q":
                            lhsT = u[:, z, jt * 128 : (jt + 1) * 128]
                            rhs = ocT[:, z * CW : z * CW + cs * 128]
                        else:
                            lhsT = oqT[:, z * KW + jt * 128 : z * KW + (jt + 1) * 128]
                            rhs = u[:, z, :]
                        nc.tensor.matmul(sps, lhsT, rhs, start=(z == 0), stop=(z == KT - 1))
                    e = wpool.tile([128, cs * 128], f16, tag=f"e{s}_{jt}", name=f"e{s}_{jt}")
                    qb = oqT[:, (KT - 1) * KW + qs * 128 + jt : (KT - 1) * KW + qs * 128 + jt + 1]
                    nc.scalar.activation(out=e, in_=sps, func=Exp, bias=qb, scale=SCALE)
                    e_tiles.append(e)
                    if jt >= 1:
                        nc.tensor.matmul(
                            dps, ones, e_tiles[jt - 1],
                            start=(jt == 1), stop=False, skip_group_check=True,
                        )
                nc.tensor.matmul(
                    dps, ones, e_tiles[qs - 1],
                    start=(qs == 1), stop=True, skip_group_check=True,
                )

                osb_tiles = {}

                def ctx_group(it, hb):
                    if it not in osb_tiles:
                        osb_tiles[it] = opool.tile([128, H], f16, tag="osb", name=f"osb{s}_{it}")
                    cps = ps_c.tile([128, 512], f32, tag="cps", name=f"cps{s}_{it}{hb}")
                    for jt in range(qs):
                        nc.tensor.matmul(
                            cps,
                            e_tiles[jt][:, it * 128 : (it + 1) * 128],
                            oqN[:, jt, hb * 512 : (hb + 1) * 512],
                            start=(jt == 0),
                            stop=(jt == qs - 1),
                        )
                    return cps

                # ship the raw denominator row to the host (host divides);
                # evict via single-partition copy then a 1-descriptor DMA
                dsb = wpool.tile([1, cs * 128], f32, tag=f"dsb{s}", name=f"dsb{s}")
                nc.vector.tensor_copy(out=dsb, in_=dps)
                nc.scalar.dma_start(out=den_d[s][:, :], in_=dsb)
                if s == 0 and B_LOCAL > 1:
                    emit_late_inputs(1)

                for it in range(cs):
                    for hb in range(2):
                        cps = ctx_group(it, hb)
                        osb = osb_tiles[it]
                        nc.vector.tensor_copy(
                            out=osb[:, hb * 512 : (hb + 1) * 512], in_=cps
                        )
                        eng = nc.sync if (2 * it + hb) % 2 == 0 else nc.scalar
                        eng.dma_start(
                            out=out_d[s][it * 128 : (it + 1) * 128, hb * 512 : (hb + 1) * 512],
                            in_=osb[:, hb * 512 : (hb + 1) * 512],
                        )
                    if it == 0 and s + 1 < B_LOCAL:
                        emit_inputs(s + 1)

    nc.compile()
    return nc


def _host_inputs(o_c, o_q, q_lengths, slots, assignment, wtQ, wtC):
    """Per-core input maps: slab packing per (core, slot) batch."""
    in_maps = [dict() for _ in range(N_CORES)]
    for c in range(N_CORES):
        if wtQ is not None:
            in_maps[c]["wtQ"] = wtQ
        if wtC is not None:
            in_maps[c]["wtC"] = wtC
    jp = np.arange(128)[:, None]  # partition index within a j-tile
    for b in range(B):
        c, s = assignment[b]
        qs, cs = slots[s]
        KW, CW = qs * 128 + 8, cs * 128 + 8
        J, I = qs * 128, cs * 128
        oq = np.asarray(o_q[b], np.float16)
        oc = np.asarray(o_c[b], np.float16)
        # oqT slab: [128, KT*KW]; col k*KW + j = oq[j, k*128+p]; qb in k=KT-1
        oqT = np.zeros((128, KT * KW), np.float16)
        t = oq[:J].T.reshape(KT, 128, J).transpose(1, 0, 2)  # [p, k, j]
        oqT.reshape(128, KT, KW)[:, :, :J] = t
        ql = int(q_lengths[b])
        qb = np.where(jp + np.arange(qs)[None, :] * 128 < ql, np.float16(0.0), np.float16(NEG16))
        oqT.reshape(128, KT, KW)[:, KT - 1, J : J + qs] = qb
        # ocT slab: [128, KT*CW]; col k*CW + i = oc[i, k*128+p]; ones col k=0
        ocT = np.zeros((128, KT * CW), np.float16)
        t = oc[:I].T.reshape(KT, 128, I).transpose(1, 0, 2)
        ocT.reshape(128, KT, CW)[:, :, :I] = t
        ocT.reshape(128, KT, CW)[:, 0, I] = 1.0
        in_maps[c][f"oqT{s}"] = oqT
        in_maps[c][f"ocT{s}"] = ocT
        in_maps[c][f"oqN{s}"] = np.ascontiguousarray(
            oq[:J].reshape(qs, 128, H).transpose(1, 0, 2)
        )
    return in_maps


def kernel(**inputs) -> np.ndarray:
    o_c = np.asarray(inputs["o_c"], dtype=np.float32)
    o_q = np.asarray(inputs["o_q"], dtype=np.float32)
    W = np.asarray(inputs["W"], dtype=np.float32)
    q_lengths = np.asarray(inputs["q_lengths"]).astype(np.int64)
    c_lengths = np.asarray(inputs["c_lengths"]).astype(np.int64)

    from concourse.bass_utils import run_bass_kernel_spmd

    qt = np.clip(np.ceil(q_lengths / 128).astype(int), 1, Tq // 128)
    ct = np.clip(np.ceil(c_lengths / 128).astype(int), 1, Tc // 128)
    slots, assignment = _plan_slots(qt, ct)
    sides = ["q" if qs <= cs else "c" for qs, cs in slots]

    Wh = W.astype(np.float16)
    wtQ = (
        np.ascontiguousarray(Wh.T.reshape(KT, 128, KT, 128).transpose(1, 2, 0, 3).reshape(128, KT * H))
        if "q" in sides else None
    )
    wtC = (
        np.ascontiguousarray(Wh.reshape(KT, 128, KT, 128).transpose(1, 2, 0, 3).reshape(128, KT * H))
        if "c" in sides else None
    )

    cache = getattr(kernel, "_cache", None)
    if cache is None:
        cache = kernel._cache = {}
    key = tuple(slots)
    if key not in cache:
        cache[key] = _build_program(slots)
    nc = cache[key]

    in_maps = _host_inputs(o_c, o_q, q_lengths, slots, assignment, wtQ, wtC)

    trace = bool(int(os.environ.get("KERNEL_TRACE", "0")))
    res = run_bass_kernel_spmd(
        nc, in_maps, core_ids=list(range(N_CORES)), trace=trace
    )
    if trace:
        kernel.last_results = res

    out = np.zeros((B, Tc, H), dtype=np.float32)
    for b in range(B):
        c, s = assignment[b]
        cl = min(int(c_lengths[b]), slots[s][1] * 128)
        den = np.maximum(res.results[c][f"den{s}"][0, :cl].astype(np.float32), 1e-30)
        out[b, :cl] = res.results[c][f"out{s}"][:cl].astype(np.float32) / den[:, None]
    return out
